# revision 2
# baseline (speedup 1.0000x reference)
"""Fused MHA-with-RoPE kernel for one TRN2 chip (8 NeuronCores).

Sharding: core c handles batch b = c//2 and head-group g = c%2 (8 of 16
heads).  Each core:
  phase 1: QKV projections (fp32r matmuls) + RoPE on q/k, q/k/v spilled to
           DRAM in attention-friendly layouts (qT/kT transposed, v natural)
  phase 2: causal attention per head, computed transposed (sT[j,i]) so no
           P transposes are needed; softmax denominator via ones-matmul;
           normalization via PE ones-broadcast + DVE multiply
  phase 3: output projection partial = av @ WoT over this core's 1024 dims,
           chunked pair-ReduceScatter to resolve the half-sum
Host: shards/transposes inputs, reassembles the RS-interleaved rows.

Self-contained: only numpy + concourse (runtime libs) + the axon boot shim.
"""

import math
import os
import sys
import types
from contextlib import ExitStack

import numpy as np

import concourse.bass as bass
import concourse.tile as tile
from concourse import bacc, mybir
from concourse.bass_utils import run_bass_kernel_spmd

# ---------------------------------------------------------------- constants
B, S, D = 4, 2048, 2048
H, HD = 16, 128
GROUPS = 2            # head groups (cores per batch)
HLOC = H // GROUPS    # heads per core = 8
E = HLOC * HD         # local qkv width = 1024
N_CORES = 8
CORE_IDS = list(range(N_CORES))
SCALE = 1.0 / math.sqrt(HD)
NEG = -1.0e30
ROPE_BASE = 10000.0

F32 = mybir.dt.float32
F32R = mybir.dt.float32r

_cache = {}


def _register_ntff_hook():
    """trn_boot can't register the NTFF profile hook (antenv.axon_hooks is
    missing from this image); recreate it so BASS_TRACE=1 profiling works."""
    if "antenv.axon_hooks" in sys.modules:
        return
    try:
        from trn_agent_boot.trn_boot import _ntff_profile_via_ctypes

        holder = {"h": _ntff_profile_via_ctypes("/opt/axon/libaxon_pjrt.so")}
        mod = types.ModuleType("antenv.axon_hooks")
        mod.get_axon_ntff_profile_hook = lambda: holder["h"]
        mod.set_axon_ntff_profile_hook = lambda h: holder.__setitem__("h", h)
        sys.modules["antenv.axon_hooks"] = mod
    except Exception:
        pass


def _host_tables():
    inv_freq = 1.0 / (ROPE_BASE ** (np.arange(0, HD, 2, dtype=np.float64) / HD))
    pos = np.arange(S, dtype=np.float64)
    freqs = pos[:, None] * inv_freq[None, :]
    emb = np.concatenate([freqs, freqs], axis=-1)        # [S, HD]
    cosT = np.ascontiguousarray(np.cos(emb).T.astype(np.float32))  # [HD, S]
    sinT = np.ascontiguousarray(np.sin(emb).T.astype(np.float32))
    sinF = sinT.copy()
    sinF[: HD // 2] *= -1.0                              # fold rotate_half sign
    return cosT, sinF


def _host_masks():
    # masks[j_local, o, i_local]: 0 if i_local >= o*128 + j_local else NEG
    m = np.empty((128, 4, 512), np.float32)
    jj = np.arange(128)[:, None]
    ii = np.arange(512)[None, :]
    for o in range(4):
        m[:, o, :] = np.where(ii >= o * 128 + jj, 0.0, NEG)
    return m


def _build_nc():
    nc = bacc.Bacc("TRN2", target_bir_lowering=False, debug=False,
                   num_devices=N_CORES)

    xT_e = nc.dram_tensor("xT", [D, S], F32R, kind="ExternalInput")
    wqT_e = nc.dram_tensor("wqT", [D, E], F32R, kind="ExternalInput")
    wkT_e = nc.dram_tensor("wkT", [D, E], F32R, kind="ExternalInput")
    wvT_e = nc.dram_tensor("wvT", [D, E], F32R, kind="ExternalInput")
    woT_e = nc.dram_tensor("woT", [E, D], F32R, kind="ExternalInput")
    out_e = nc.dram_tensor("out", [4, 512 // GROUPS, D], F32,
                           kind="ExternalOutput")

    cosT_d = nc.inline_tensor(_host_tables()[0], name="cosT")
    sinF_d = nc.inline_tensor(_host_tables()[1], name="sinF")
    masks_d = nc.inline_tensor(_host_masks(), name="masks")
    ones_col_d = nc.inline_tensor(np.ones((128, 1), np.float32), name="ones_col")
    ones_row_d = nc.inline_tensor(np.ones((1, 128), np.float32), name="ones_row")

    with tile.TileContext(nc) as tc, ExitStack() as ctx:
        dram = ctx.enter_context(tc.tile_pool(name="dram", bufs=1, space="DRAM"))
        qT_d = dram.tile([E, S], F32R)
        kT_d = dram.tile([E, S], F32R)
        v_d = dram.tile([S, E], F32R)
        partial_d = dram.tile([S, D], F32)
        rs_out_d = dram.tile([4, 512 // GROUPS, D], F32)

        consts = ctx.enter_context(tc.tile_pool(name="consts", bufs=1))
        cos_sb = consts.tile([HD, S], F32)
        sinF_sb = consts.tile([HD, S], F32)
        masks_sb = consts.tile([128, 4, 512], F32)
        ones_col = consts.tile([128, 1], F32R)
        ones_row = consts.tile([1, 128], F32R)
        nc.gpsimd.dma_start(out=cos_sb[:], in_=cosT_d[:])
        nc.gpsimd.dma_start(out=sinF_sb[:], in_=sinF_d[:])
        nc.gpsimd.dma_start(out=masks_sb[:], in_=masks_d[:])
        nc.gpsimd.dma_start(out=ones_col[:], in_=ones_col_d[:])
        nc.gpsimd.dma_start(out=ones_row[:], in_=ones_row_d[:])

        HF = HD // 2

        # ---------------- phase 1: projections ----------------
        with tc.tile_pool(name="xT", bufs=1) as xT_pool:
            xT_sb = xT_pool.tile([128, 16, S], F32R)
            nc.gpsimd.dma_start(
                out=xT_sb[:], in_=xT_e[:].rearrange("(t p) s -> p t s", p=128))

            # q/k projections + RoPE, spilled transposed [E, S]
            with tc.tile_pool(name="wqk", bufs=2) as wqk_pool, \
                 tc.tile_pool(name="rope_wk", bufs=4) as rwk, \
                 tc.tile_pool(name="rot_out", bufs=4) as rout, \
                 tc.tile_pool(name="ps1", bufs=4, space="PSUM") as ps1:
                for w_e, o_d, pname in ((wqT_e, qT_d, "q"), (wkT_e, kT_d, "k")):
                    for m in range(HLOC):
                        w_sb = wqk_pool.tile([128, 16, 128], F32R,
                                             name=f"w{pname}{m}", tag="w")
                        nc.gpsimd.dma_start(
                            out=w_sb[:],
                            in_=w_e[:, bass.ts(m, 128)].rearrange(
                                "(t p) e -> p t e", p=128))
                        for sb in range(4):
                            ps = ps1.tile([128, 512], F32, name="ps_qk",
                                          tag="ps_qk")
                            for dt_ in range(16):
                                nc.tensor.matmul(
                                    ps[:], w_sb[:, dt_, :],
                                    xT_sb[:, dt_, bass.ts(sb, 512)],
                                    start=(dt_ == 0), stop=(dt_ == 15))
                            c_sl = cos_sb[:, bass.ts(sb, 512)]
                            s_sl = sinF_sb[:, bass.ts(sb, 512)]
                            sw = rwk.tile([128, 512], F32, name="sw", tag="sw")
                            nc.vector.tensor_copy(sw[0:HF, :], ps[HF:HD, :])
                            nc.vector.tensor_copy(sw[HF:HD, :], ps[0:HF, :])
                            m1 = rwk.tile([128, 512], F32, name="m1", tag="m1")
                            nc.vector.tensor_mul(m1[:], ps[:], c_sl)
                            m2 = rwk.tile([128, 512], F32, name="m2", tag="m2")
                            nc.vector.tensor_mul(m2[:], sw[:], s_sl)
                            rot = rout.tile([128, 512], F32R, name="rot",
                                            tag="rot")
                            nc.vector.tensor_add(rot[:], m1[:], m2[:])
                            nc.gpsimd.dma_start(
                                out=o_d[bass.ts(m, 128), bass.ts(sb, 512)],
                                in_=rot[:])

            # v projection, natural layout [S, E]
            with tc.tile_pool(name="wv", bufs=1) as wv_pool, \
                 tc.tile_pool(name="vout", bufs=4) as vout, \
                 tc.tile_pool(name="ps1b", bufs=4, space="PSUM") as ps1b:
                for n in range(2):
                    wv_sb = wv_pool.tile([128, 16, 512], F32R, name=f"wv{n}",
                                         tag="wv")
                    nc.gpsimd.dma_start(
                        out=wv_sb[:],
                        in_=wvT_e[:, bass.ts(n, 512)].rearrange(
                            "(t p) e -> p t e", p=128))
                    for st in range(16):
                        ps = ps1b.tile([128, 512], F32, name="ps_v", tag="ps_v")
                        for dt_ in range(16):
                            nc.tensor.matmul(
                                ps[:], xT_sb[:, dt_, bass.ts(st, 128)],
                                wv_sb[:, dt_, :],
                                start=(dt_ == 0), stop=(dt_ == 15))
                        vt = vout.tile([128, 512], F32R, name="vt", tag="vt")
                        nc.vector.tensor_copy(vt[:], ps[:])
                        nc.gpsimd.dma_start(
                            out=v_d[bass.ts(st, 128), bass.ts(n, 512)],
                            in_=vt[:])

        # ---------------- phase 2: attention ----------------
        avT_pool = ctx.enter_context(tc.tile_pool(name="avT", bufs=1))
        avT_sb = avT_pool.tile([128, HLOC, S], F32R)

        with tc.tile_pool(name="qh", bufs=2) as qh_pool, \
             tc.tile_pool(name="kh", bufs=2) as kh_pool, \
             tc.tile_pool(name="vh", bufs=2) as vh_pool, \
             tc.tile_pool(name="wk2", bufs=6) as wk2, \
             tc.tile_pool(name="ps2", bufs=2, space="PSUM") as ps2, \
             tc.tile_pool(name="psacc", bufs=2, space="PSUM") as psacc:
            for h in range(HLOC):
                q_sb = qh_pool.tile([128, S], F32R, name=f"qh{h}", tag="qh")
                nc.gpsimd.dma_start(out=q_sb[:], in_=qT_d[bass.ts(h, 128), :])
                k_sb = kh_pool.tile([128, S], F32R, name=f"kh{h}", tag="kh")
                nc.gpsimd.dma_start(out=k_sb[:], in_=kT_d[bass.ts(h, 128), :])
                v_sb = vh_pool.tile([128, 16, 128], F32R, name=f"vh{h}",
                                    tag="vh")
                nc.gpsimd.dma_start(
                    out=v_sb[:],
                    in_=v_d[:, bass.ts(h, 128)].rearrange(
                        "(t p) d -> p t d", p=128))
                for ib in range(4):
                    nj = 4 * (ib + 1)
                    den_ps = psacc.tile([1, 512], F32, name="den", tag="den")
                    av_ps = psacc.tile([128, 512], F32, name="av", tag="av")
                    for jt in range(nj):
                        s_ps = ps2.tile([128, 512], F32, name="s_ps",
                                        tag="s_ps")
                        nc.tensor.matmul(s_ps[:], k_sb[:, bass.ts(jt, 128)],
                                         q_sb[:, bass.ts(ib, 512)],
                                         start=True, stop=True)
                        o_diag = jt - 4 * ib
                        if o_diag >= 0:
                            msk = wk2.tile([128, 512], F32, name="msk",
                                           tag="msk")
                            nc.vector.tensor_add(msk[:], s_ps[:],
                                                 masks_sb[:, o_diag, :])
                            src = msk
                        else:
                            src = s_ps
                        pT = wk2.tile([128, 512], F32R, name="pT", tag="pT")
                        nc.scalar.activation(
                            pT[:], src[:], mybir.ActivationFunctionType.Exp,
                            scale=SCALE)
                        nc.tensor.matmul(den_ps[:], ones_col[:], pT[:],
                                         start=(jt == 0), stop=(jt == nj - 1))
                        nc.tensor.matmul(av_ps[:], v_sb[:, jt, :], pT[:],
                                         start=(jt == 0), stop=(jt == nj - 1))
                    rden = wk2.tile([1, 512], F32R, name="rden", tag="rden")
                    with nc.allow_low_precision(reason="f32r rounding only"):
                        nc.vector.reciprocal(rden[:], den_ps[:])
                    bc_ps = ps2.tile([128, 512], F32, name="bc_ps", tag="bc")
                    nc.tensor.matmul(bc_ps[:], ones_row[:], rden[:],
                                     start=True, stop=True)
                    bc_sb = wk2.tile([128, 512], F32, name="bc_sb", tag="bcs")
                    nc.vector.tensor_copy(bc_sb[:], bc_ps[:])
                    nc.vector.tensor_mul(avT_sb[:, h, bass.ts(ib, 512)],
                                         av_ps[:], bc_sb[:])

        # ---------------- phase 3: Wo partial + chunked pair-RS ----------
        with tc.tile_pool(name="wo", bufs=1) as wo_pool, \
             tc.tile_pool(name="out3", bufs=4) as out3, \
             tc.tile_pool(name="ps3", bufs=4, space="PSUM") as ps3:
            wo_sb = wo_pool.tile([128, HLOC, D], F32R)
            nc.gpsimd.dma_start(
                out=wo_sb[:], in_=woT_e[:].rearrange("(t p) e -> p t e", p=128))
            for im in range(16):
                for eb in range(4):
                    ps = ps3.tile([128, 512], F32, name="ps_o", tag="ps_o")
                    for hh in range(HLOC):
                        nc.tensor.matmul(ps[:],
                                         avT_sb[:, hh, bass.ts(im, 128)],
                                         wo_sb[:, hh, bass.ts(eb, 512)],
                                         start=(hh == 0), stop=(hh == HLOC - 1))
                    po = out3.tile([128, 512], F32, name="po", tag="po")
                    nc.vector.tensor_copy(po[:], ps[:])
                    nc.gpsimd.dma_start(
                        out=partial_d[bass.ts(im, 128), bass.ts(eb, 512)],
                        in_=po[:])
                if im % 4 == 3:
                    ch = im // 4
                    nc.gpsimd.collective_compute(
                        "ReduceScatter",
                        mybir.AluOpType.add,
                        replica_groups=[[0, 1], [2, 3], [4, 5], [6, 7]],
                        ins=[partial_d[ch * 512:(ch + 1) * 512, :]],
                        outs=[rs_out_d[ch]],
                    )
            nc.gpsimd.dma_start(out=out_e[:], in_=rs_out_d[:])

    nc.compile()
    return nc


def kernel(x, Wq, Wk, Wv, Wo):
    _register_ntff_hook()
    if "nc" not in _cache:
        _cache["nc"] = _build_nc()
    nc = _cache["nc"]

    in_maps = []
    for c in CORE_IDS:
        b, g = c // GROUPS, c % GROUPS
        sl = slice(g * E, (g + 1) * E)
        in_maps.append({
            "xT": np.ascontiguousarray(x[b].T),
            "wqT": np.ascontiguousarray(Wq[sl, :].T),
            "wkT": np.ascontiguousarray(Wk[sl, :].T),
            "wvT": np.ascontiguousarray(Wv[sl, :].T),
            "woT": np.ascontiguousarray(Wo[:, sl].T),
        })

    trace = bool(os.environ.get("BASS_TRACE"))
    res = run_bass_kernel_spmd(nc, in_maps, CORE_IDS, trace=trace)
    kernel.last_exec_time_ns = res.exec_time_ns

    out = np.empty((B, S, D), np.float32)
    half = 512 // GROUPS
    for c in CORE_IDS:
        b, g = c // GROUPS, c % GROUPS
        r = res.results[c]["out"]          # [4, 256, D]
        for ch in range(4):
            lo = ch * 512 + g * half
            out[b, lo:lo + half, :] = r[ch]
    return out


kernel.last_exec_time_ns = None
